# revision 30
# baseline (speedup 1.0000x reference)
"""Trainium2 Bass kernel for naive causal MHA (dense transformer block).

Problem: x[2, 2048, 1024], per-head QKV (16 heads, head_dim 64), causal
softmax attention, concat heads, output projection.

Sharding (8 NeuronCores, tensor-parallel over heads):
  - core c computes QKV + attention for heads {2c, 2c+1} over both batches
    in a transposed layout: scores are built as [keys, queries] so the
    softmax denominator comes from an extra ones-column in V and the
    attention output lands directly in the [head_dim, seq] layout the
    output projection needs as its stationary operand.
  - one 8-way AllToAll PER BATCH reshards y from head-split to row-split
    (the batch-0 collective and output projection overlap batch-1 compute),
  - each core computes a disjoint 256-row slice of y @ Wout + bout per batch.

Perf notes vs the f32r baseline (450 us -> ~250 us):
  - all matmuls in bf16 (f32r moving operands stream at half rate),
  - per-head q/k tiles zero-padded to 128 contraction rows and V padded to
    128 stationary cols: sub-128 attention matmuls otherwise keep the PE
    activity monitor at K=4/8 (1.2 GHz instead of 2.4),
  - exp over [128, 2, 512] per t-block (both heads, 2 PSUM banks, double
    buffered) to amortize the ~352-cycle ACTIVATE overhead and pipeline
    ACT against the PE,
  - causal trim: diagonal t-blocks only compute columns >= j*128,
  - softmax normalization: one PSUM->SBUF copy frees psy, then a ones
    matmul broadcasts z and reciprocal_approx_fast runs off-critical-path,
  - QKV chains are popped as fillers between a t-block's score and AV
    matmuls (exp latency hides them); out-proj(b0) pops into the tail of
    batch-1 attention once a2a(0) has landed,
  - bulk DMAs ride the gpsimd queue so small attention-critical DMAs are
    never head-of-line blocked on the sync queue.
"""

import contextlib
import ctypes
import sys
import types

import numpy as np

import concourse.bacc as bacc
import concourse.mybir as mybir
import concourse.tile as tile
from concourse.bass import ds

N_CORES = 8
B = 2
S = 2048
D = 1024
HD = 64
N_HEADS = 16

F32 = mybir.dt.float32
DTB = mybir.dt.bfloat16
NP_BF16 = mybir.dt.np(mybir.dt.bfloat16)

SC = 512          # seq chunk (moving-operand width)
N_SC = S // SC    # 4
N_DC = D // 128   # 8 contraction chunks
N_SB = S // 128   # 16 seq 128-blocks
CQ = S // N_CORES // B  # 128... no: per-batch a2a slot width = 2048/8 = 256
CQ = S // N_CORES       # 256 q per a2a slot


def _f32r(ap):
    return ap.bitcast(mybir.dt.float32r)


def _mask_np():
    """Upper-triangular keep-mask (t <= q) for the diagonal 128x128 score
    block, duplicated for both heads: [t, h, q]."""
    tri = np.triu(np.ones((128, 128), dtype=np.float32))
    return np.stack([tri, tri], axis=1).astype(NP_BF16)


def _build_program():
    nc = bacc.Bacc(
        "TRN2", target_bir_lowering=False, debug=False, num_devices=N_CORES
    )

    xt_d = nc.dram_tensor("xt", [B, N_SC, N_DC, 128, SC], DTB, kind="ExternalInput").ap()
    wq_d = nc.dram_tensor("wq", [D, 128], DTB, kind="ExternalInput").ap()
    wk_d = nc.dram_tensor("wk", [D, 128], DTB, kind="ExternalInput").ap()
    wv_d = nc.dram_tensor("wv", [D, 128], DTB, kind="ExternalInput").ap()
    bq_d = nc.dram_tensor("bq", [128, 1], F32, kind="ExternalInput").ap()
    bk_d = nc.dram_tensor("bk", [128, 1], F32, kind="ExternalInput").ap()
    bv_d = nc.dram_tensor("bv", [128, 1], F32, kind="ExternalInput").ap()
    wout_d = nc.dram_tensor("wout", [D, D], DTB, kind="ExternalInput").ap()
    bout_d = nc.dram_tensor("bout", [1, D], F32, kind="ExternalInput").ap()
    out_d = nc.dram_tensor("out", [B, 2 * 128, D], F32, kind="ExternalOutput").ap()

    y_part = [nc.dram_tensor(f"y{b}p", [N_CORES, 128, CQ], DTB) for b in range(B)]
    y_all = [nc.dram_tensor(f"y{b}a", [N_CORES, 128, CQ], DTB) for b in range(B)]

    mask_d = nc.inline_tensor(_mask_np(), name="tri")
    ones64_d = nc.inline_tensor(np.ones((1, 64), dtype=NP_BF16), name="ones64")
    eye_d = nc.inline_tensor(np.eye(128, dtype=np.float32).astype(NP_BF16), name="eye128")

    with tile.TileContext(nc) as tc, contextlib.ExitStack() as ctx:
        const = ctx.enter_context(tc.tile_pool(name="const", bufs=1))
        xt_pool = ctx.enter_context(tc.tile_pool(name="xt", bufs=1))
        qk_pool = ctx.enter_context(tc.tile_pool(name="qk", bufs=1))
        v_pool = ctx.enter_context(tc.tile_pool(name="vp", bufs=1))
        exp_pool = ctx.enter_context(tc.tile_pool(name="expp", bufs=2))
        r_pool = ctx.enter_context(tc.tile_pool(name="rp", bufs=2))
        z_pool = ctx.enter_context(tc.tile_pool(name="zp", bufs=2))
        yts_pool = ctx.enter_context(tc.tile_pool(name="yts", bufs=3))
        qs_pool = ctx.enter_context(tc.tile_pool(name="qs", bufs=2))
        yg_pool = ctx.enter_context(tc.tile_pool(name="yg", bufs=1))
        outs_pool = ctx.enter_context(tc.tile_pool(name="outs", bufs=2))
        psum = ctx.enter_context(tc.tile_pool(name="psum", bufs=1, space="PSUM"))

        # ---- constants into SBUF (critical-path order: wq + biases
        # before the bulk; slow partition-broadcast loads on gpsimd) ----
        wq_sb = const.tile([128, N_DC, 128], DTB)
        nc.sync.dma_start(out=wq_sb, in_=wq_d.rearrange("(c p) e -> p c e", p=128))
        bq_sb = const.tile([128, 1], F32)
        bk_sb = const.tile([128, 1], F32)
        wk_sb = const.tile([128, N_DC, 128], DTB)
        wv_sb = const.tile([128, N_DC, 128], DTB)
        tri_sb = const.tile([128, 2, 128], DTB)
        ones64_sb = const.tile([1, 64], DTB)
        wout_sb = const.tile([128, N_DC, D], DTB)
        bout_bc = const.tile([128, D], F32)
        bv_sb = const.tile([128, 1], F32)
        nc.sync.dma_start(out=bv_sb, in_=bv_d)
        eye_sb = const.tile([128, 128], DTB)

        # ---- per-batch persistent SBUF ----
        xt_sb = []
        qTz, kTz, v_sb = [], [], []
        for b in range(B):
            xt_sb.append(xt_pool.tile([128, N_DC, S], DTB, tag=f"xt{b}", name=f"xt{b}"))
            qTz.append([
                qk_pool.tile([128, S], DTB, tag=f"qTz{b}{h}", name=f"qTz{b}{h}")
                for h in range(2)
            ])
            kTz.append([
                qk_pool.tile([128, S], DTB, tag=f"kTz{b}{h}", name=f"kTz{b}{h}")
                for h in range(2)
            ])
            v_sb.append(v_pool.tile([128, N_SB, 2, 128], DTB, tag=f"v{b}", name=f"v{b}"))

        def zero_pads(b):
            # whole-tile zero via int32 bitcast (fast 4x memset); QKV fills
            # rows/cols 0:64 later. Rows 64:128 of q/k stay zero so the
            # full-width contraction keeps the PE activity monitor happy;
            # v col 64 = ones (softmax denominator), 65:128 stay zero.
            for h in range(2):
                nc.vector.memset(qTz[b][h].bitcast(mybir.dt.int32), 0)
                nc.vector.memset(kTz[b][h].bitcast(mybir.dt.int32), 0)
            nc.vector.memset(
                v_sb[b].rearrange("p a h e -> p (a h e)").bitcast(mybir.dt.int32), 0
            )
            nc.vector.memset(
                v_sb[b].rearrange("p a h e -> p (a h) e")[:, :, 64:65], 1.0
            )

        def load_xt_sc(b, sc, eng=None, eng2=None):
            for dc in range(N_DC):
                e = eng or nc.gpsimd
                if eng2 is not None and dc % 2 == 1:
                    e = eng2
                e.dma_start(
                    out=xt_sb[b][:, dc, ds(sc * SC, SC)],
                    in_=xt_d[b, sc, dc],
                )

        def _qk_proj(b, sc, w_sb, bias_sb, dstz, scratch_tag):
            ps = psum.tile([128, SC], F32, tag="misc", bufs=2, name=f"ps_{scratch_tag}")
            for dc in range(N_DC):
                nc.tensor.matmul(
                    ps, w_sb[:, dc, :], xt_sb[b][:, dc, ds(sc * SC, SC)],
                    start=(dc == 0), stop=(dc == N_DC - 1),
                )
            nc.vector.tensor_scalar_add(
                out=dstz[0][0:64, ds(sc * SC, SC)], in0=ps[0:64, :],
                scalar1=bias_sb[0:64],
            )
            qs = qs_pool.tile([128, SC], DTB, tag=scratch_tag, name=f"qs_{scratch_tag}")
            nc.vector.tensor_scalar_add(
                out=qs[64:128, :], in0=ps[64:128, :], scalar1=bias_sb[64:128]
            )
            nc.sync.dma_start(
                out=dstz[1][0:64, ds(sc * SC, SC)], in_=qs[64:128, :]
            )

        def qkv_q(b, sc):
            _qk_proj(b, sc, wq_sb, bq_sb, qTz[b], "qsq")

        def qkv_k(b, sc):
            _qk_proj(b, sc, wk_sb, bk_sb, kTz[b], "qsk")

        def qkv_v(b, sc):
            # vT = Wv.T @ x (wide moving operand, LDW hidden), then
            # transpose each 128-block back to [seq, vdim] via an
            # identity matmul (values are exact: bf16 * 1.0)
            psv = psum.tile([128, SC], F32, tag="misc", bufs=2, name="psvT")
            for dc in range(N_DC):
                nc.tensor.matmul(
                    psv, wv_sb[:, dc, :], xt_sb[b][:, dc, ds(sc * SC, SC)],
                    start=(dc == 0), stop=(dc == N_DC - 1),
                )
            vts = qs_pool.tile([128, SC], DTB, tag="vts", name="vts")
            nc.vector.tensor_scalar_add(out=vts, in0=psv, scalar1=bv_sb)
            ptr = psum.tile([128, SC], F32, tag="misc", bufs=2, name="ptrv")
            ptr4 = ptr.rearrange("p (j e) -> p j e", j=4)
            for j4 in range(4):
                nc.tensor.matmul(
                    ptr4[:, j4, :], vts[:, ds(j4 * 128, 128)], eye_sb,
                    start=True, stop=True,
                )
            pjhe = ptr.rearrange("p (j h e) -> p j h e", j=4, h=2)
            for h in range(2):
                nc.vector.tensor_copy(
                    out=v_sb[b][:, ds(4 * sc, 4), h, 0:64],
                    in_=pjhe[:, :, h, :],
                )

        def attn_qc(b, qc, pop_filler):
            ntb = 4 * qc + 4
            psy = [
                psum.tile([128, SC], F32, tag="psy", bufs=2, name=f"psy{b}_{qc}_{_}")
                for _ in range(2)
            ]
            for tb in range(ntb):
                j = tb - 4 * qc
                # diagonal blocks only need columns >= j*128 (t <= q)
                o = j * 128 if j > 0 else 0
                w = SC - o
                psc = psum.tile([128, 2, SC], F32, tag="sc2", bufs=2)
                ex = exp_pool.tile([128, 2, SC], DTB)
                for h in range(2):
                    nc.tensor.matmul(
                        psc[:, h, ds(o, w)],
                        kTz[b][h][:, ds(tb * 128, 128)],
                        qTz[b][h][:, ds(qc * SC + o, w)],
                        start=True, stop=True,
                    )
                pop_filler(tb)
                nc.scalar.activation(
                    out=ex[:, :, ds(o, w)], in_=psc[:, :, ds(o, w)],
                    func=mybir.ActivationFunctionType.Exp,
                    scale=0.125,
                )
                if j >= 0:
                    nc.vector.tensor_mul(
                        out=ex[:, :, ds(j * 128, 128)],
                        in0=ex[:, :, ds(j * 128, 128)],
                        in1=tri_sb,
                    )
                for h in range(2):
                    nc.tensor.matmul(
                        psy[h][:, ds(o, w)], v_sb[b][:, tb, h, :],
                        ex[:, h, ds(o, w)],
                        start=(tb == 0), stop=(tb == ntb - 1),
                        skip_group_check=True,
                    )
            # normalize: one fast PSUM->SBUF copy per head releases psy;
            # the divide chain then runs entirely off the critical path
            for h in range(2):
                yhat = r_pool.tile([64, SC], DTB, name=f"yhat{h}")
                nc.vector.tensor_copy(out=yhat, in_=psy[h][0:64, :])
                zrow = r_pool.tile([1, SC], DTB, name=f"zrow{h}")
                nc.vector.tensor_copy(out=zrow, in_=psy[h][64:65, :])
                zb = psum.tile([128, SC], F32, tag="misc", bufs=2)
                nc.tensor.matmul(
                    zb[0:64, :], ones64_sb, zrow, start=True, stop=True
                )
                rbc = z_pool.tile([64, SC], F32)
                nc.vector.reciprocal_approx_fast(out=rbc, in_=zb[0:64, :])
                yts = yts_pool.tile([64, SC], DTB)
                nc.vector.tensor_mul(out=yts, in0=yhat, in1=rbc)
                nc.sync.dma_start(
                    out=y_part[b].ap()[ds(2 * qc, 2), ds(64 * h, 64), :].transpose(
                        [1, 0, 2]
                    ),
                    in_=yts.rearrange("p (c q) -> p c q", c=2),
                )

        ygs = [yg_pool.tile([128, N_CORES, CQ], DTB, tag=f"yg{b}", name=f"yg{b}") for b in range(B)]

        def load_ygs(b):
            for ec in range(N_CORES):
                nc.sync.dma_start(out=ygs[b][:, ec, :], in_=y_all[b].ap()[ec])

        def outproj_piece(b, qb):
            pso = psum.tile(
                [128, 2, SC], F32, tag="sc2", bufs=2, name=f"pso{b}{qb}"
            )
            # ec outer / ch inner: each ygs stationary is loaded once and
            # reused for both 512-col halves, so LDWEIGHTS hides under the
            # previous matmul instead of serializing
            for ec in range(N_CORES):
                for ch in range(2):
                    nc.tensor.matmul(
                        pso[:, ch, :],
                        ygs[b][:, ec, ds(qb * 128, 128)],
                        wout_sb[:, ec, ds(ch * SC, SC)],
                        start=(ec == 0), stop=(ec == N_CORES - 1),
                        skip_group_check=True,
                    )
            for ch in range(2):
                ot = outs_pool.tile([128, SC], F32, name=f"ot{ch}")
                nc.vector.tensor_add(
                    out=ot, in0=pso[:, ch, :], in1=bout_bc[:, ds(ch * SC, SC)]
                )
                nc.sync.dma_start(
                    out=out_d[b, ds(qb * 128, 128), ds(ch * SC, SC)], in_=ot
                )

        def a2a(b):
            nc.gpsimd.collective_compute(
                "AllToAll",
                mybir.AluOpType.bypass,
                replica_groups=[list(range(N_CORES))],
                ins=[y_part[b].ap()],
                outs=[y_all[b].ap()],
            )

        # ================= emission =================
        # xt(b0,sc0) on the latency-critical sync queue; all other bulk
        # loads go through the (otherwise idle) gpsimd queue so small
        # attention-critical DMAs are never stuck behind them
        load_xt_sc(0, 0, eng=nc.sync, eng2=nc.scalar)
        load_xt_sc(0, 1)
        nc.gpsimd.dma_start(out=eye_sb, in_=eye_d.ap())
        nc.gpsimd.dma_start(out=tri_sb, in_=mask_d.ap())
        nc.gpsimd.dma_start(out=ones64_sb, in_=ones64_d.ap())
        nc.sync.dma_start(out=bq_sb, in_=bq_d)
        nc.sync.dma_start(out=bk_sb, in_=bk_d)
        nc.sync.dma_start(out=wk_sb, in_=wk_d.rearrange("(c p) e -> p c e", p=128))
        nc.sync.dma_start(out=wv_sb, in_=wv_d.rearrange("(c p) e -> p c e", p=128))
        zero_pads(0)
        for sc in range(1, N_SC):
            load_xt_sc(0, sc)
        qkv_q(0, 0)
        qkv_k(0, 0)
        qkv_v(0, 0)
        zero_pads(1)
        for sc in range(N_SC):
            load_xt_sc(1, sc)
        nc.gpsimd.dma_start(
            out=wout_sb, in_=wout_d.rearrange("(c p) e -> p c e", p=128)
        )
        nc.gpsimd.dma_start(out=bout_bc, in_=bout_d.to_broadcast([128, D]))

        # remaining QKV work as an ordered unit queue; units with key
        # (b, sc) must be emitted before attn_qc(b, qc >= sc)
        units = []
        for key in [(0, 1), (0, 2), (0, 3), (1, 0), (1, 1), (1, 2), (1, 3)]:
            b, sc = key
            units.append((key, lambda b=b, sc=sc: qkv_q(b, sc)))
            units.append((key, lambda b=b, sc=sc: qkv_k(b, sc)))
            units.append((key, lambda b=b, sc=sc: qkv_v(b, sc)))

        def flush_to(key):
            while units and units[0][0] <= key:
                units.pop(0)[1]()

        def popper(limit_key, late_key=None):
            # late_key units are only popped in the back half of a chunk's
            # t-block sweep (used to gate work behind an in-flight a2a)
            def pop(tb):
                if not units:
                    return
                key = units[0][0]
                if key <= limit_key or (
                    late_key is not None and key <= late_key and tb >= 8
                ):
                    units.pop(0)[1]()
            return pop

        for qc in range(N_SC):
            flush_to((0, qc))
            attn_qc(0, qc, popper((1, 0)))
        flush_to((1, 0))
        a2a(0)

        for qc in range(N_SC):
            flush_to((1, qc))
            if qc == N_SC - 1:
                units.append(((2, 0), lambda: load_ygs(0)))
                units.append(((2, 0), lambda: outproj_piece(0, 0)))
                units.append(((2, 0), lambda: outproj_piece(0, 1)))
            attn_qc(1, qc, popper((1, 3), late_key=(2, 0)))
        flush_to((2, 0))
        a2a(1)

        # keep the PE clock un-throttled through the a2a(1) wait; one
        # stationary load, outputs are never read. ~10.5us of filler is
        # still below the minimum observed a2a latency, so these never
        # delay the real out-projection.
        for w in range(5):
            trash = psum.tile([128, SC], F32, tag="misc", bufs=2, name=f"warm{w}")
            for m in range(8):
                nc.tensor.matmul(
                    trash, kTz[1][0][:, 0:128],
                    qTz[1][0][:, ds((m % 4) * 128, SC)],
                    start=(m == 0), stop=(m == 7),
                )

        load_ygs(1)
        for qb in range(2):
            outproj_piece(1, qb)

    nc.compile()
    return nc


_NC_CACHE = None


def _get_program():
    global _NC_CACHE
    if _NC_CACHE is None:
        _NC_CACHE = _build_program()
    return _NC_CACHE


def make_in_maps(x, Wqkv, bqkv, Wout, bout):
    x = np.asarray(x, dtype=np.float32)
    Wqkv = np.asarray(Wqkv, dtype=np.float32)
    bqkv = np.asarray(bqkv, dtype=np.float32)
    Wout = np.asarray(Wout, dtype=np.float32)
    bout = np.asarray(bout, dtype=np.float32)

    # [B, D, S] -> chunk-major [B, sc, dc, 128, SC]: per-(sc,dc) loads
    # are fully contiguous 128KB and spread across DMA rings
    xt = (
        x.transpose(0, 2, 1)
        .reshape(B, N_DC, 128, N_SC, SC)
        .transpose(0, 3, 1, 2, 4)
    )
    xt = np.ascontiguousarray(xt).astype(NP_BF16)
    wout = np.ascontiguousarray(Wout).astype(NP_BF16)
    bout2 = np.ascontiguousarray(bout.reshape(1, D))

    in_maps = []
    for c in range(N_CORES):
        h0, h1 = 2 * c, 2 * c + 1
        wq = np.concatenate(
            [Wqkv[h0, :, 0:64], Wqkv[h1, :, 0:64]], axis=1
        ).astype(NP_BF16)
        wk = np.concatenate(
            [Wqkv[h0, :, 64:128], Wqkv[h1, :, 64:128]], axis=1
        ).astype(NP_BF16)
        wv = np.concatenate(
            [Wqkv[h0, :, 128:192], Wqkv[h1, :, 128:192]], axis=1
        ).astype(NP_BF16)
        bq = np.ascontiguousarray(
            np.concatenate([bqkv[h0, 0:64], bqkv[h1, 0:64]]).reshape(128, 1)
        )
        bk = np.ascontiguousarray(
            np.concatenate([bqkv[h0, 64:128], bqkv[h1, 64:128]]).reshape(128, 1)
        )
        bv = np.ascontiguousarray(
            np.concatenate([bqkv[h0, 128:192], bqkv[h1, 128:192]]).reshape(128, 1)
        )
        in_maps.append(
            {
                "xt": xt,
                "wq": np.ascontiguousarray(wq),
                "wk": np.ascontiguousarray(wk),
                "wv": np.ascontiguousarray(wv),
                "bq": bq,
                "bk": bk,
                "bv": bv,
                "wout": wout,
                "bout": bout2,
            }
        )
    return in_maps


def assemble(results):
    full = np.empty((B, S, D), dtype=np.float32)
    for c in range(N_CORES):
        full[:, 256 * c : 256 * (c + 1)] = results[c]["out"]
    return full


def _install_ntff_hook():
    """The agent image's antenv lacks axon_hooks; provide it so
    run_bass_kernel_spmd(trace=True) can NTFF-profile via libaxon."""
    if "antenv.axon_hooks" in sys.modules:
        return
    so_path = "/opt/axon/libaxon_pjrt.so"
    try:
        lib = ctypes.CDLL(so_path)
        lib.axon_start_nrt_profile.argtypes = [
            ctypes.POINTER(ctypes.c_int64),
            ctypes.c_size_t,
        ]
        lib.axon_start_nrt_profile.restype = ctypes.c_int64
        lib.axon_stop_nrt_profile.argtypes = [ctypes.c_char_p]
        lib.axon_stop_nrt_profile.restype = ctypes.c_int64
    except (OSError, AttributeError):
        return

    @contextlib.contextmanager
    def _hook(output_dir, device_ids):
        import jax

        jax.devices()
        if device_ids:
            ids = (ctypes.c_int64 * len(device_ids))(*device_ids)
            rc = lib.axon_start_nrt_profile(ids, len(device_ids))
        else:
            rc = lib.axon_start_nrt_profile(None, 0)
        if rc != 0:
            raise RuntimeError(f"axon_start_nrt_profile rc={rc}")
        try:
            yield
        finally:
            n = lib.axon_stop_nrt_profile(str(output_dir).encode())
            if n < 0:
                raise RuntimeError(f"axon_stop_nrt_profile rc={n}")

    mod = types.ModuleType("antenv.axon_hooks")
    mod.get_axon_ntff_profile_hook = lambda: _hook
    mod.set_axon_ntff_profile_hook = lambda h: None
    sys.modules["antenv.axon_hooks"] = mod


def run(inputs, trace=False):
    """Run on the 8 NeuronCores. Returns (output, BassKernelResults)."""
    from concourse.bass_utils import run_bass_kernel_spmd

    if trace:
        _install_ntff_hook()
    nc = _get_program()
    in_maps = make_in_maps(**inputs)
    res = run_bass_kernel_spmd(
        nc, in_maps, core_ids=list(range(N_CORES)), trace=trace
    )
    return assemble(res.results), res


def kernel(x, Wqkv, bqkv, Wout, bout):
    out, _ = run(
        {"x": x, "Wqkv": Wqkv, "bqkv": bqkv, "Wout": Wout, "bout": bout},
        trace=False,
    )
    return out


# revision 31
# speedup vs baseline: 1.1416x; 1.1416x over previous
"""Trainium2 Bass kernel for naive causal MHA (dense transformer block).

Problem: x[2, 2048, 1024], per-head QKV (16 heads, head_dim 64), causal
softmax attention, concat heads, output projection.

Sharding (8 NeuronCores, tensor-parallel over heads):
  - core c computes QKV + attention for heads {2c, 2c+1} over both batches
    in a transposed layout: scores are built as [keys, queries] so the
    softmax denominator comes from an extra ones-column in V and the
    attention output lands directly in the [head_dim, seq] layout the
    output projection needs as its stationary operand.
  - one 8-way AllToAll PER BATCH reshards y from head-split to row-split
    (the batch-0 collective and output projection overlap batch-1 compute),
  - each core computes a disjoint 256-row slice of y @ Wout + bout per batch.

Perf notes vs the f32r baseline (450 us -> ~250 us):
  - all matmuls in bf16 (f32r moving operands stream at half rate),
  - per-head q/k tiles zero-padded to 128 contraction rows and V padded to
    128 stationary cols: sub-128 attention matmuls otherwise keep the PE
    activity monitor at K=4/8 (1.2 GHz instead of 2.4),
  - exp over [128, 2, 512] per t-block (both heads, 2 PSUM banks, double
    buffered) to amortize the ~352-cycle ACTIVATE overhead and pipeline
    ACT against the PE,
  - causal trim: diagonal t-blocks only compute columns >= j*128,
  - softmax normalization: one PSUM->SBUF copy frees psy, then a ones
    matmul broadcasts z and reciprocal_approx_fast runs off-critical-path,
  - QKV chains are popped as fillers between a t-block's score and AV
    matmuls (exp latency hides them); out-proj(b0) pops into the tail of
    batch-1 attention once a2a(0) has landed,
  - bulk DMAs ride the gpsimd queue so small attention-critical DMAs are
    never head-of-line blocked on the sync queue.
"""

import contextlib
import ctypes
import sys
import types

import numpy as np

import concourse.bacc as bacc
import concourse.mybir as mybir
import concourse.tile as tile
from concourse.bass import ds

N_CORES = 8
B = 2
S = 2048
D = 1024
HD = 64
N_HEADS = 16

F32 = mybir.dt.float32
DTB = mybir.dt.bfloat16
NP_BF16 = mybir.dt.np(mybir.dt.bfloat16)

SC = 512          # seq chunk (moving-operand width)
N_SC = S // SC    # 4
N_DC = D // 128   # 8 contraction chunks
N_SB = S // 128   # 16 seq 128-blocks
CQ = S // N_CORES // B  # 128... no: per-batch a2a slot width = 2048/8 = 256
CQ = S // N_CORES       # 256 q per a2a slot


def _f32r(ap):
    return ap.bitcast(mybir.dt.float32r)


def _mask_np():
    """Upper-triangular keep-mask (t <= q) for the diagonal 128x128 score
    block, duplicated for both heads: [t, h, q]."""
    tri = np.triu(np.ones((128, 128), dtype=np.float32))
    return np.stack([tri, tri], axis=1).astype(NP_BF16)


def _build_program():
    nc = bacc.Bacc(
        "TRN2", target_bir_lowering=False, debug=False, num_devices=N_CORES
    )

    xt_d = nc.dram_tensor("xt", [B, N_SC, N_DC, 128, SC], DTB, kind="ExternalInput").ap()
    wq_d = nc.dram_tensor("wq", [D, 128], DTB, kind="ExternalInput").ap()
    wk_d = nc.dram_tensor("wk", [D, 128], DTB, kind="ExternalInput").ap()
    wv_d = nc.dram_tensor("wv", [D, 128], DTB, kind="ExternalInput").ap()
    bq_d = nc.dram_tensor("bq", [128, 1], F32, kind="ExternalInput").ap()
    bk_d = nc.dram_tensor("bk", [128, 1], F32, kind="ExternalInput").ap()
    bv_d = nc.dram_tensor("bv", [128, 1], F32, kind="ExternalInput").ap()
    wout_d = nc.dram_tensor("wout", [D, D], DTB, kind="ExternalInput").ap()
    bout_d = nc.dram_tensor("bout", [1, D], F32, kind="ExternalInput").ap()
    out_d = nc.dram_tensor("out", [B, 2 * 128, D], F32, kind="ExternalOutput").ap()

    y_part = [nc.dram_tensor(f"y{b}p", [N_CORES, 128, CQ], DTB) for b in range(B)]
    y_all = [nc.dram_tensor(f"y{b}a", [N_CORES, 128, CQ], DTB) for b in range(B)]

    mask_d = nc.inline_tensor(_mask_np(), name="tri")
    ones64_d = nc.inline_tensor(np.ones((1, 64), dtype=NP_BF16), name="ones64")
    eye_d = nc.inline_tensor(np.eye(128, dtype=np.float32).astype(NP_BF16), name="eye128")

    with tile.TileContext(nc) as tc, contextlib.ExitStack() as ctx:
        const = ctx.enter_context(tc.tile_pool(name="const", bufs=1))
        xt_pool = ctx.enter_context(tc.tile_pool(name="xt", bufs=1))
        qk_pool = ctx.enter_context(tc.tile_pool(name="qk", bufs=1))
        v_pool = ctx.enter_context(tc.tile_pool(name="vp", bufs=1))
        exp_pool = ctx.enter_context(tc.tile_pool(name="expp", bufs=2))
        r_pool = ctx.enter_context(tc.tile_pool(name="rp", bufs=2))
        z_pool = ctx.enter_context(tc.tile_pool(name="zp", bufs=2))
        yts_pool = ctx.enter_context(tc.tile_pool(name="yts", bufs=3))
        qs_pool = ctx.enter_context(tc.tile_pool(name="qs", bufs=2))
        yg_pool = ctx.enter_context(tc.tile_pool(name="yg", bufs=1))
        outs_pool = ctx.enter_context(tc.tile_pool(name="outs", bufs=2))
        psum = ctx.enter_context(tc.tile_pool(name="psum", bufs=1, space="PSUM"))

        # ---- constants into SBUF (critical-path order: wq + biases
        # before the bulk; slow partition-broadcast loads on gpsimd) ----
        wq_sb = const.tile([128, N_DC, 128], DTB)
        nc.sync.dma_start(out=wq_sb, in_=wq_d.rearrange("(c p) e -> p c e", p=128))
        bq_sb = const.tile([128, 1], F32)
        bk_sb = const.tile([128, 1], F32)
        wk_sb = const.tile([128, N_DC, 128], DTB)
        wv_sb = const.tile([128, N_DC, 128], DTB)
        tri_sb = const.tile([128, 2, 128], DTB)
        ones64_sb = const.tile([1, 64], DTB)
        wout_sb = const.tile([128, N_DC, D], DTB)
        bout_bc = const.tile([128, D], F32)
        bv_sb = const.tile([128, 1], F32)
        nc.sync.dma_start(out=bv_sb, in_=bv_d)
        eye_sb = const.tile([128, 128], DTB)

        # ---- per-batch persistent SBUF ----
        xt_sb = []
        qTz, kTz, v_sb = [], [], []
        for b in range(B):
            xt_sb.append(xt_pool.tile([128, N_DC, S], DTB, tag=f"xt{b}", name=f"xt{b}"))
            qTz.append([
                qk_pool.tile([128, S], DTB, tag=f"qTz{b}{h}", name=f"qTz{b}{h}")
                for h in range(2)
            ])
            kTz.append([
                qk_pool.tile([128, S], DTB, tag=f"kTz{b}{h}", name=f"kTz{b}{h}")
                for h in range(2)
            ])
            v_sb.append(v_pool.tile([128, N_SB, 2, 128], DTB, tag=f"v{b}", name=f"v{b}"))

        def zero_pads(b, scalar_eng=False):
            # whole-tile zero; QKV fills rows/cols 0:64 later. Rows 64:128
            # of q/k stay zero so the full-width contraction keeps the PE
            # activity monitor happy; v col 64 = ones (softmax denom),
            # 65:128 stay zero. For batch 0 the big zero-fills run on the
            # idle scalar engine so the DVE can start the QKV bias-adds
            # (attention's startup dependency) immediately.
            for h in range(2):
                if scalar_eng:
                    nc.scalar.memzero(qTz[b][h])
                    nc.scalar.memzero(kTz[b][h])
                else:
                    nc.vector.memset(qTz[b][h].bitcast(mybir.dt.int32), 0)
                    nc.vector.memset(kTz[b][h].bitcast(mybir.dt.int32), 0)
            vflat = v_sb[b].rearrange("p a h e -> p (a h e)")
            if scalar_eng:
                nc.scalar.memzero(vflat)
            else:
                nc.vector.memset(vflat.bitcast(mybir.dt.int32), 0)
            nc.vector.memset(
                v_sb[b].rearrange("p a h e -> p (a h) e")[:, :, 64:65], 1.0
            )

        def load_xt_sc(b, sc, eng=None, eng2=None):
            for dc in range(N_DC):
                e = eng or nc.gpsimd
                if eng2 is not None and dc % 2 == 1:
                    e = eng2
                e.dma_start(
                    out=xt_sb[b][:, dc, ds(sc * SC, SC)],
                    in_=xt_d[b, sc, dc],
                )

        def _qk_proj(b, sc, w_sb, bias_sb, dstz, scratch_tag):
            ps = psum.tile([128, SC], F32, tag="misc", bufs=2, name=f"ps_{scratch_tag}")
            for dc in range(N_DC):
                nc.tensor.matmul(
                    ps, w_sb[:, dc, :], xt_sb[b][:, dc, ds(sc * SC, SC)],
                    start=(dc == 0), stop=(dc == N_DC - 1),
                )
            nc.vector.tensor_scalar_add(
                out=dstz[0][0:64, ds(sc * SC, SC)], in0=ps[0:64, :],
                scalar1=bias_sb[0:64],
            )
            qs = qs_pool.tile([128, SC], DTB, tag=scratch_tag, name=f"qs_{scratch_tag}")
            nc.vector.tensor_scalar_add(
                out=qs[64:128, :], in0=ps[64:128, :], scalar1=bias_sb[64:128]
            )
            nc.sync.dma_start(
                out=dstz[1][0:64, ds(sc * SC, SC)], in_=qs[64:128, :]
            )

        def qkv_q(b, sc):
            _qk_proj(b, sc, wq_sb, bq_sb, qTz[b], "qsq")

        def qkv_k(b, sc):
            _qk_proj(b, sc, wk_sb, bk_sb, kTz[b], "qsk")

        def qkv_v(b, sc):
            # vT = Wv.T @ x (wide moving operand, LDW hidden), then
            # transpose each 128-block back to [seq, vdim] via an
            # identity matmul (values are exact: bf16 * 1.0)
            psv = psum.tile([128, SC], F32, tag="misc", bufs=2, name="psvT")
            for dc in range(N_DC):
                nc.tensor.matmul(
                    psv, wv_sb[:, dc, :], xt_sb[b][:, dc, ds(sc * SC, SC)],
                    start=(dc == 0), stop=(dc == N_DC - 1),
                )
            vts = qs_pool.tile([128, SC], DTB, tag="vts", name="vts")
            nc.vector.tensor_scalar_add(out=vts, in0=psv, scalar1=bv_sb)
            ptr = psum.tile([128, SC], F32, tag="misc", bufs=2, name="ptrv")
            ptr4 = ptr.rearrange("p (j e) -> p j e", j=4)
            for j4 in range(4):
                nc.tensor.matmul(
                    ptr4[:, j4, :], vts[:, ds(j4 * 128, 128)], eye_sb,
                    start=True, stop=True,
                )
            pjhe = ptr.rearrange("p (j h e) -> p j h e", j=4, h=2)
            for h in range(2):
                nc.vector.tensor_copy(
                    out=v_sb[b][:, ds(4 * sc, 4), h, 0:64],
                    in_=pjhe[:, :, h, :],
                )

        def attn_qc(b, qc, pop_filler):
            ntb = 4 * qc + 4
            psy = [
                psum.tile([128, SC], F32, tag="psy", bufs=2, name=f"psy{b}_{qc}_{_}")
                for _ in range(2)
            ]
            for tb in range(ntb):
                j = tb - 4 * qc
                # diagonal blocks only need columns >= j*128 (t <= q)
                o = j * 128 if j > 0 else 0
                w = SC - o
                psc = psum.tile([128, 2, SC], F32, tag="sc2", bufs=2)
                ex = exp_pool.tile([128, 2, SC], DTB)
                for h in range(2):
                    nc.tensor.matmul(
                        psc[:, h, ds(o, w)],
                        kTz[b][h][:, ds(tb * 128, 128)],
                        qTz[b][h][:, ds(qc * SC + o, w)],
                        start=True, stop=True,
                    )
                pop_filler(tb)
                nc.scalar.activation(
                    out=ex[:, :, ds(o, w)], in_=psc[:, :, ds(o, w)],
                    func=mybir.ActivationFunctionType.Exp,
                    scale=0.125,
                )
                if j >= 0:
                    nc.vector.tensor_mul(
                        out=ex[:, :, ds(j * 128, 128)],
                        in0=ex[:, :, ds(j * 128, 128)],
                        in1=tri_sb,
                    )
                for h in range(2):
                    nc.tensor.matmul(
                        psy[h][:, ds(o, w)], v_sb[b][:, tb, h, :],
                        ex[:, h, ds(o, w)],
                        start=(tb == 0), stop=(tb == ntb - 1),
                        skip_group_check=True,
                    )
            # normalize: one fast PSUM->SBUF copy per head releases psy;
            # the divide chain then runs entirely off the critical path
            for h in range(2):
                yhat = r_pool.tile([64, SC], DTB, name=f"yhat{h}")
                nc.vector.tensor_copy(out=yhat, in_=psy[h][0:64, :])
                zrow = r_pool.tile([1, SC], DTB, name=f"zrow{h}")
                nc.vector.tensor_copy(out=zrow, in_=psy[h][64:65, :])
                zb = psum.tile([128, SC], F32, tag="misc", bufs=2)
                nc.tensor.matmul(
                    zb[0:64, :], ones64_sb, zrow, start=True, stop=True
                )
                rbc = z_pool.tile([64, SC], F32)
                nc.vector.reciprocal_approx_fast(out=rbc, in_=zb[0:64, :])
                yts = yts_pool.tile([64, SC], DTB)
                nc.vector.tensor_mul(out=yts, in0=yhat, in1=rbc)
                nc.sync.dma_start(
                    out=y_part[b].ap()[ds(2 * qc, 2), ds(64 * h, 64), :].transpose(
                        [1, 0, 2]
                    ),
                    in_=yts.rearrange("p (c q) -> p c q", c=2),
                )

        ygs = [yg_pool.tile([128, N_CORES, CQ], DTB, tag=f"yg{b}", name=f"yg{b}") for b in range(B)]

        def load_ygs(b):
            for ec in range(N_CORES):
                nc.sync.dma_start(out=ygs[b][:, ec, :], in_=y_all[b].ap()[ec])

        def outproj_piece(b, qb):
            pso = psum.tile(
                [128, 2, SC], F32, tag="sc2", bufs=2, name=f"pso{b}{qb}"
            )
            # ec outer / ch inner: each ygs stationary is loaded once and
            # reused for both 512-col halves, so LDWEIGHTS hides under the
            # previous matmul instead of serializing
            for ec in range(N_CORES):
                for ch in range(2):
                    nc.tensor.matmul(
                        pso[:, ch, :],
                        ygs[b][:, ec, ds(qb * 128, 128)],
                        wout_sb[:, ec, ds(ch * SC, SC)],
                        start=(ec == 0), stop=(ec == N_CORES - 1),
                        skip_group_check=True,
                    )
            for ch in range(2):
                ot = outs_pool.tile([128, SC], F32, name=f"ot{ch}")
                nc.vector.tensor_add(
                    out=ot, in0=pso[:, ch, :], in1=bout_bc[:, ds(ch * SC, SC)]
                )
                nc.sync.dma_start(
                    out=out_d[b, ds(qb * 128, 128), ds(ch * SC, SC)], in_=ot
                )

        def a2a(b):
            nc.gpsimd.collective_compute(
                "AllToAll",
                mybir.AluOpType.bypass,
                replica_groups=[list(range(N_CORES))],
                ins=[y_part[b].ap()],
                outs=[y_all[b].ap()],
            )

        # ================= emission =================
        # xt(b0,sc0) on the latency-critical sync queue; all other bulk
        # loads go through the (otherwise idle) gpsimd queue so small
        # attention-critical DMAs are never stuck behind them
        load_xt_sc(0, 0, eng=nc.sync, eng2=nc.scalar)
        load_xt_sc(0, 1)
        nc.gpsimd.dma_start(out=eye_sb, in_=eye_d.ap())
        nc.gpsimd.dma_start(out=tri_sb, in_=mask_d.ap())
        nc.gpsimd.dma_start(out=ones64_sb, in_=ones64_d.ap())
        nc.sync.dma_start(out=bq_sb, in_=bq_d)
        nc.sync.dma_start(out=bk_sb, in_=bk_d)
        nc.sync.dma_start(out=wk_sb, in_=wk_d.rearrange("(c p) e -> p c e", p=128))
        nc.sync.dma_start(out=wv_sb, in_=wv_d.rearrange("(c p) e -> p c e", p=128))
        zero_pads(0, scalar_eng=True)
        for sc in range(1, N_SC):
            load_xt_sc(0, sc)
        qkv_q(0, 0)
        qkv_k(0, 0)
        qkv_v(0, 0)
        zero_pads(1)
        for sc in range(N_SC):
            load_xt_sc(1, sc)
        nc.gpsimd.dma_start(
            out=wout_sb, in_=wout_d.rearrange("(c p) e -> p c e", p=128)
        )
        nc.gpsimd.dma_start(out=bout_bc, in_=bout_d.to_broadcast([128, D]))

        # remaining QKV work as an ordered unit queue; units with key
        # (b, sc) must be emitted before attn_qc(b, qc >= sc)
        units = []
        for key in [(0, 1), (0, 2), (0, 3), (1, 0), (1, 1), (1, 2), (1, 3)]:
            b, sc = key
            units.append((key, lambda b=b, sc=sc: qkv_q(b, sc)))
            units.append((key, lambda b=b, sc=sc: qkv_k(b, sc)))
            units.append((key, lambda b=b, sc=sc: qkv_v(b, sc)))

        def flush_to(key):
            while units and units[0][0] <= key:
                units.pop(0)[1]()

        def popper(limit_key, late_key=None):
            # late_key units are only popped in the back half of a chunk's
            # t-block sweep (used to gate work behind an in-flight a2a)
            def pop(tb):
                if not units:
                    return
                key = units[0][0]
                if key <= limit_key or (
                    late_key is not None and key <= late_key and tb >= 8
                ):
                    units.pop(0)[1]()
            return pop

        for qc in range(N_SC):
            flush_to((0, qc))
            attn_qc(0, qc, popper((1, 0)))
        flush_to((1, 0))
        a2a(0)

        for qc in range(N_SC):
            flush_to((1, qc))
            if qc == N_SC - 1:
                units.append(((2, 0), lambda: load_ygs(0)))
                units.append(((2, 0), lambda: outproj_piece(0, 0)))
                units.append(((2, 0), lambda: outproj_piece(0, 1)))
            attn_qc(1, qc, popper((1, 3), late_key=(2, 0)))
        flush_to((2, 0))
        a2a(1)

        # keep the PE clock un-throttled through the a2a(1) wait; one
        # stationary load, outputs are never read. ~10.5us of filler is
        # still below the minimum observed a2a latency, so these never
        # delay the real out-projection.
        for w in range(5):
            trash = psum.tile([128, SC], F32, tag="misc", bufs=2, name=f"warm{w}")
            for m in range(8):
                nc.tensor.matmul(
                    trash, kTz[1][0][:, 0:128],
                    qTz[1][0][:, ds((m % 4) * 128, SC)],
                    start=(m == 0), stop=(m == 7),
                )

        load_ygs(1)
        for qb in range(2):
            outproj_piece(1, qb)

    nc.compile()
    return nc


_NC_CACHE = None


def _get_program():
    global _NC_CACHE
    if _NC_CACHE is None:
        _NC_CACHE = _build_program()
    return _NC_CACHE


def make_in_maps(x, Wqkv, bqkv, Wout, bout):
    x = np.asarray(x, dtype=np.float32)
    Wqkv = np.asarray(Wqkv, dtype=np.float32)
    bqkv = np.asarray(bqkv, dtype=np.float32)
    Wout = np.asarray(Wout, dtype=np.float32)
    bout = np.asarray(bout, dtype=np.float32)

    # [B, D, S] -> chunk-major [B, sc, dc, 128, SC]: per-(sc,dc) loads
    # are fully contiguous 128KB and spread across DMA rings
    xt = (
        x.transpose(0, 2, 1)
        .reshape(B, N_DC, 128, N_SC, SC)
        .transpose(0, 3, 1, 2, 4)
    )
    xt = np.ascontiguousarray(xt).astype(NP_BF16)
    wout = np.ascontiguousarray(Wout).astype(NP_BF16)
    bout2 = np.ascontiguousarray(bout.reshape(1, D))

    in_maps = []
    for c in range(N_CORES):
        h0, h1 = 2 * c, 2 * c + 1
        wq = np.concatenate(
            [Wqkv[h0, :, 0:64], Wqkv[h1, :, 0:64]], axis=1
        ).astype(NP_BF16)
        wk = np.concatenate(
            [Wqkv[h0, :, 64:128], Wqkv[h1, :, 64:128]], axis=1
        ).astype(NP_BF16)
        wv = np.concatenate(
            [Wqkv[h0, :, 128:192], Wqkv[h1, :, 128:192]], axis=1
        ).astype(NP_BF16)
        bq = np.ascontiguousarray(
            np.concatenate([bqkv[h0, 0:64], bqkv[h1, 0:64]]).reshape(128, 1)
        )
        bk = np.ascontiguousarray(
            np.concatenate([bqkv[h0, 64:128], bqkv[h1, 64:128]]).reshape(128, 1)
        )
        bv = np.ascontiguousarray(
            np.concatenate([bqkv[h0, 128:192], bqkv[h1, 128:192]]).reshape(128, 1)
        )
        in_maps.append(
            {
                "xt": xt,
                "wq": np.ascontiguousarray(wq),
                "wk": np.ascontiguousarray(wk),
                "wv": np.ascontiguousarray(wv),
                "bq": bq,
                "bk": bk,
                "bv": bv,
                "wout": wout,
                "bout": bout2,
            }
        )
    return in_maps


def assemble(results):
    full = np.empty((B, S, D), dtype=np.float32)
    for c in range(N_CORES):
        full[:, 256 * c : 256 * (c + 1)] = results[c]["out"]
    return full


def _install_ntff_hook():
    """The agent image's antenv lacks axon_hooks; provide it so
    run_bass_kernel_spmd(trace=True) can NTFF-profile via libaxon."""
    if "antenv.axon_hooks" in sys.modules:
        return
    so_path = "/opt/axon/libaxon_pjrt.so"
    try:
        lib = ctypes.CDLL(so_path)
        lib.axon_start_nrt_profile.argtypes = [
            ctypes.POINTER(ctypes.c_int64),
            ctypes.c_size_t,
        ]
        lib.axon_start_nrt_profile.restype = ctypes.c_int64
        lib.axon_stop_nrt_profile.argtypes = [ctypes.c_char_p]
        lib.axon_stop_nrt_profile.restype = ctypes.c_int64
    except (OSError, AttributeError):
        return

    @contextlib.contextmanager
    def _hook(output_dir, device_ids):
        import jax

        jax.devices()
        if device_ids:
            ids = (ctypes.c_int64 * len(device_ids))(*device_ids)
            rc = lib.axon_start_nrt_profile(ids, len(device_ids))
        else:
            rc = lib.axon_start_nrt_profile(None, 0)
        if rc != 0:
            raise RuntimeError(f"axon_start_nrt_profile rc={rc}")
        try:
            yield
        finally:
            n = lib.axon_stop_nrt_profile(str(output_dir).encode())
            if n < 0:
                raise RuntimeError(f"axon_stop_nrt_profile rc={n}")

    mod = types.ModuleType("antenv.axon_hooks")
    mod.get_axon_ntff_profile_hook = lambda: _hook
    mod.set_axon_ntff_profile_hook = lambda h: None
    sys.modules["antenv.axon_hooks"] = mod


def run(inputs, trace=False):
    """Run on the 8 NeuronCores. Returns (output, BassKernelResults)."""
    from concourse.bass_utils import run_bass_kernel_spmd

    if trace:
        _install_ntff_hook()
    nc = _get_program()
    in_maps = make_in_maps(**inputs)
    res = run_bass_kernel_spmd(
        nc, in_maps, core_ids=list(range(N_CORES)), trace=trace
    )
    return assemble(res.results), res


def kernel(x, Wqkv, bqkv, Wout, bout):
    out, _ = run(
        {"x": x, "Wqkv": Wqkv, "bqkv": bqkv, "Wout": Wout, "bout": bout},
        trace=False,
    )
    return out
